# revision 10
# baseline (speedup 1.0000x reference)
"""Trainium2 Bass kernel for the E74 checkpointed delta-rule cell.

Math (per batch b):
    k,v,q = x @ W_{k,v,q}^T            # [T, N] each
    kn = k / (||k|| + 1e-6)
    scan over t: r = S kn_t ; S += outer(v_t - r, kn_t) ; z = S q_t
                 out_t = z * silu(z)

Chunked parallel form (chunk C=128, per batch, chunk index c):
    A  = stril(KN KN^T)   (strict lower, within chunk)
    T  = (I + A)^{-1}     (unit lower triangular inverse, Newton iterations)
    U  = T (V - KN S0^T)
    O  = Q S0^T + (tril_incl(Q KN^T)) U
    S1^T = S0^T + KN^T U
    out = O * silu(O)

Sharding: data parallel over batch B=32 -> 8 cores x 4 batches.

Engine routing: PE does all matmuls/transposes (fp32r for big ones, bf16
for the inverse iteration, fp32 for the final Newton refinement); DVE
drains PSUM-consuming elementwise; ACT does only Copy-drains (phase A)
and Silu (phase B) to avoid act-table reloads; GPSIMD does SBUF-only
elementwise (mask builds, norm squares, Newton rsqrt for 1/(||k||+eps)).
"""

from contextlib import ExitStack

import numpy as np

import concourse.bacc as bacc
import concourse.mybir as mybir
import concourse.tile as tile
from concourse import masks
from concourse.bass_utils import run_bass_kernel_spmd

F32 = mybir.dt.float32
F32R = mybir.dt.float32r
BF16 = mybir.dt.bfloat16
I32 = mybir.dt.int32
AT = mybir.ActivationFunctionType
OP = mybir.AluOpType

T, B, DIM, N = 512, 32, 1024, 256
NCORES = 8
BL = B // NCORES  # local batches per core = 4
C = 128  # chunk length
NCH = T // C  # chunks per batch = 4
NT = N // 128  # n tiles = 2
DT = DIM // 128  # d tiles = 8
P3N = 3 * N  # packed projection width (k|v|q)

# knobs
N_BF16_ITERS = 1  # bf16 Newton iterations (order 2^(iters+1)-1 Neumann terms)
REFINE_FP32 = True  # final Newton step in fp32
RSQRT_ITERS = 3  # Newton iterations for 1/sqrt on gpsimd
REPS = 1


def build_body(nc, tc, ctx, xT, wT, s0T, out):
    consts = ctx.enter_context(tc.tile_pool(name="consts", bufs=1))
    wpool = ctx.enter_context(tc.tile_pool(name="w", bufs=1))
    xpool = ctx.enter_context(tc.tile_pool(name="x", bufs=8))
    kvq = ctx.enter_context(tc.tile_pool(name="kvq", bufs=1))
    vtp = ctx.enter_context(tc.tile_pool(name="vt", bufs=2))
    state = ctx.enter_context(tc.tile_pool(name="state", bufs=1))
    pre = ctx.enter_context(tc.tile_pool(name="pre", bufs=16))
    dbl = ctx.enter_context(tc.tile_pool(name="dbl", bufs=3))
    seq = ctx.enter_context(tc.tile_pool(name="seq", bufs=4))
    nrm = ctx.enter_context(tc.tile_pool(name="nrm", bufs=2))
    proj_ps = ctx.enter_context(tc.tile_pool(name="proj_ps", bufs=2, space="PSUM"))
    work_ps = ctx.enter_context(tc.tile_pool(name="work_ps", bufs=6, space="PSUM"))

    # ---- constants ----
    ident = consts.tile([128, 128], F32, tag="ident")
    masks.make_identity(nc, ident[:])
    ident_bf = consts.tile([128, 128], BF16, tag="identb")
    nc.gpsimd.tensor_copy(ident_bf[:], ident[:])
    ident2 = consts.tile([128, 128], F32, tag="ident2")
    masks.make_identity(nc, ident2[:])
    nc.gpsimd.tensor_scalar_mul(ident2[:], ident2[:], 2.0)
    mask_negL = consts.tile([128, 128], F32, tag="mnl")  # -1 on strict lower
    masks.make_lower_triangular(nc, mask_negL[:], val=-1.0, diag=False)
    mask_negU = consts.tile([128, 128], F32, tag="mnu")  # -1 on strict upper
    masks.make_upper_triangular(nc, mask_negU[:], val=-1.0, diag=False)
    mask_U_excl = consts.tile([128, 128], F32, tag="mue")  # 1 on strict upper
    masks.make_upper_triangular(nc, mask_U_excl[:], val=1.0, diag=False)
    mask_U_incl = consts.tile([128, 128], F32, tag="mui")  # 1 on upper+diag
    masks.make_upper_triangular(nc, mask_U_incl[:], val=1.0, diag=True)
    ident_r = consts.tile([128, 128], F32R, tag="identr")
    nc.gpsimd.tensor_copy(ident_r[:], ident[:])
    ones_col_f = consts.tile([128, 1], F32, tag="onescf")
    nc.gpsimd.memset(ones_col_f[:], 1.0)
    ones_col = consts.tile([128, 1], F32R, tag="onesc")
    nc.gpsimd.tensor_copy(ones_col[:], ones_col_f[:])
    ones_row_f = consts.tile([1, 128], F32, tag="onesrf")
    nc.gpsimd.memset(ones_row_f[:], 1.0)
    ones_row = consts.tile([1, 128], F32R, tag="onesr")
    nc.gpsimd.tensor_copy(ones_row[:], ones_row_f[:])

    # weights resident: 8 tiles [128, 768]
    w_sb = []
    for d in range(DT):
        wt = wpool.tile([128, P3N], F32R, tag=f"w{d}")
        nc.sync.dma_start(wt[:], wT[d * 128 : (d + 1) * 128, :])
        w_sb.append(wt)

    for rep in range(REPS):
        STm = [[None] * NT for _ in range(BL)]
        KTm = [[None] * NT for _ in range(BL)]
        QTm = [[None] * NT for _ in range(BL)]
        TTm = [[None] * NCH for _ in range(BL)]
        GTm = [[None] * NCH for _ in range(BL)]
        Knat = [[None] * NCH for _ in range(BL)]
        Vnat = [[None] * NCH for _ in range(BL)]

        # =========== phase A: projections + chunk precompute, per batch ====
        for b in range(BL):
            for jt in range(NT):
                st = state.tile([128, N], F32R, tag=f"st{b}{jt}")
                nc.sync.dma_start(st[:], s0T[b, jt * 128 : (jt + 1) * 128, :])
                STm[b][jt] = st

            xt = []
            for d in range(DT):
                x_sb = xpool.tile([128, T], F32R, tag="x")
                nc.sync.dma_start(x_sb[:], xT[d * 128 : (d + 1) * 128, b, :])
                xt.append(x_sb)

            # projections: out[n, t] = sum_d wT[d, n] * xT[d, t]
            KT, VT, QT = [None] * NT, [None] * NT, [None] * NT
            for p in range(3):  # k, v, q
                for nt in range(NT):
                    ps = proj_ps.tile([128, T], F32, tag="proj")
                    col0 = p * N + nt * 128
                    for d in range(DT):
                        nc.tensor.matmul(
                            ps[:],
                            w_sb[d][:, col0 : col0 + 128],
                            xt[d][:],
                            start=(d == 0),
                            stop=(d == DT - 1),
                        )
                    if p == 1:
                        dst = vtp.tile([128, T], F32R, tag=f"vt{nt}")
                    else:
                        dst = kvq.tile([128, T], F32R, tag=f"kvq{p}{nt}{b}")
                    if p == 0:
                        nc.vector.tensor_copy(dst[:], ps[:])  # k: DVE
                    else:
                        nc.scalar.copy(dst[:], ps[:])  # v, q: ACT
                    [KT, VT, QT][p][nt] = dst
            KTm[b], QTm[b] = KT, QT

            # ---- kn = k * rsqrt(ssq) (+eps folded; see note) ----
            # ssq[1, t] = sum_n k^2 via ones-vector matmul
            ssq_ps = work_ps.tile([1, T], F32, tag="ps")
            for nt in range(NT):
                sq = nrm.tile([128, T], F32R, tag="sq")
                nc.gpsimd.tensor_mul(sq[:], KT[nt][:], KT[nt][:])
                nc.tensor.matmul(
                    ssq_ps[:], ones_col[:], sq[:], start=(nt == 0), stop=(nt == NT - 1)
                )
            ssq = nrm.tile([1, T], F32, tag="ssq")
            nc.vector.tensor_copy(ssq[:], ssq_ps[:])
            # y0 = bit-trick seed; Newton y <- y*(1.5 - 0.5*x*y^2)
            yi = nrm.tile([1, T], I32, tag="yi")
            nc.vector.tensor_scalar(
                yi[:], ssq[:].bitcast(I32), 1, None, op0=OP.arith_shift_right
            )
            nc.vector.tensor_scalar_mul(yi[:], yi[:], -1)
            nc.vector.tensor_scalar_add(yi[:], yi[:], 0x5F3759DF)
            y = nrm.tile([1, T], F32, tag="y")
            nc.gpsimd.tensor_copy(y[:], yi[:].bitcast(F32))
            a = nrm.tile([1, T], F32, tag="a")
            for _ in range(RSQRT_ITERS):
                nc.gpsimd.tensor_mul(a[:], y[:], y[:])
                nc.gpsimd.tensor_mul(a[:], a[:], ssq[:])
                nc.gpsimd.tensor_scalar_mul(a[:], a[:], -0.5)
                nc.gpsimd.tensor_scalar_add(a[:], a[:], 1.5)
                nc.gpsimd.tensor_mul(y[:], y[:], a[:])
            # eps correction: 1/(s+eps) ~= y*(1 - eps*y); eps=1e-6, s~16
            nc.gpsimd.tensor_scalar_mul(a[:], y[:], -1e-6)
            nc.gpsimd.tensor_scalar_add(a[:], a[:], 1.0)
            inv = nrm.tile([1, T], F32R, tag="inv")
            nc.gpsimd.tensor_mul(inv[:], y[:], a[:])
            # broadcast along partitions via rank-1 matmul, then normalize k
            bc_ps = work_ps.tile([128, T], F32, tag="ps")
            nc.tensor.matmul(bc_ps[:], ones_row[:], inv[:], start=True, stop=True)
            bc = nrm.tile([128, T], F32, tag="bc")
            nc.scalar.copy(bc[:], bc_ps[:])
            for nt in range(NT):
                nc.gpsimd.tensor_mul(KT[nt][:], KT[nt][:], bc[:])

            # ---- per chunk precompute ----
            for c in range(NCH):
                cs = slice(c * C, (c + 1) * C)
                pw0 = c if c < NCH - 1 else c - 1
                off = 0 if c < NCH - 1 else 128
                pws = slice(pw0 * C, (pw0 + 2) * C)

                # K, V natural via PE transpose
                kn_t = pre.tile([128, N], F32R, tag="knat")
                vn_t = pre.tile([128, N], F32R, tag="vnat")
                for nt in range(NT):
                    tp = work_ps.tile([128, 128], F32R, tag="ps")
                    nc.tensor.transpose(tp[:], KT[nt][:, cs], ident_r[:])
                    nc.scalar.copy(kn_t[:, nt * 128 : (nt + 1) * 128], tp[:])
                    tp2 = work_ps.tile([128, 128], F32R, tag="ps")
                    nc.tensor.transpose(tp2[:], VT[nt][:, cs], ident_r[:])
                    nc.scalar.copy(vn_t[:, nt * 128 : (nt + 1) * 128], tp2[:])
                Knat[b][c] = kn_t
                Vnat[b][c] = vn_t

                # KK chunk-diagonal block (wide free=256)
                kk_ps = work_ps.tile([128, 2 * C], F32, tag="ps")
                for nt in range(NT):
                    nc.tensor.matmul(
                        kk_ps[:],
                        KT[nt][:, cs],
                        KT[nt][:, pws],
                        start=(nt == 0),
                        stop=(nt == NT - 1),
                    )
                kkb = dbl.tile([128, 128], F32, tag="kkb")
                nc.vector.tensor_copy(kkb[:], kk_ps[:, off : off + 128])

                # masked builds on gpsimd (SBUF only)
                lup = dbl.tile([128, 128], F32, tag="lup")
                nc.gpsimd.tensor_mul(lup[:], kkb[:], mask_U_excl[:])
                nc.gpsimd.tensor_add(lup[:], lup[:], ident[:])
                lup_bf = dbl.tile([128, 128], BF16, tag="lupb")
                nc.gpsimd.tensor_copy(lup_bf[:], lup[:])
                xb = dbl.tile([128, 128], BF16, tag="xb")
                nc.gpsimd.tensor_mul(xb[:], kkb[:], mask_negL[:])
                nc.gpsimd.tensor_add(xb[:], xb[:], ident_bf[:])
                xtb = dbl.tile([128, 128], BF16, tag="xtb")
                nc.gpsimd.tensor_mul(xtb[:], kkb[:], mask_negU[:])
                nc.gpsimd.tensor_add(xtb[:], xtb[:], ident_bf[:])

                # bf16 Newton: X <- X(2I - M X), tracking X and X^T
                for _ in range(N_BF16_ITERS):
                    mx = work_ps.tile([128, 128], F32, tag="ps")
                    nc.tensor.matmul(mx[:], lup_bf[:], xb[:], start=True, stop=True)
                    t2 = dbl.tile([128, 128], BF16, tag="t2")
                    nc.vector.scalar_tensor_tensor(
                        t2[:], mx[:], -1.0, ident2[:], op0=OP.mult, op1=OP.add
                    )
                    xps = work_ps.tile([128, 128], F32, tag="ps")
                    nc.tensor.matmul(xps[:], xtb[:], t2[:], start=True, stop=True)
                    xtps = work_ps.tile([128, 128], F32, tag="ps")
                    nc.tensor.matmul(xtps[:], t2[:], xtb[:], start=True, stop=True)
                    xb = dbl.tile([128, 128], BF16, tag="xb")
                    nc.vector.tensor_copy(xb[:], xps[:])
                    xtb = dbl.tile([128, 128], BF16, tag="xtb")
                    nc.vector.tensor_copy(xtb[:], xtps[:])

                tt = pre.tile([128, 128], F32R, tag="tt")
                if REFINE_FP32:
                    # fp32 Newton step; only T^T = (2I - M X)^T X^T needed
                    x32 = dbl.tile([128, 128], F32, tag="x32")
                    nc.gpsimd.tensor_copy(x32[:], xb[:])
                    xt32 = dbl.tile([128, 128], F32, tag="xt32")
                    nc.gpsimd.tensor_copy(xt32[:], xtb[:])
                    mx = work_ps.tile([128, 128], F32, tag="ps")
                    nc.tensor.matmul(mx[:], lup[:], x32[:], start=True, stop=True)
                    t2f = dbl.tile([128, 128], F32, tag="t2f")
                    nc.vector.scalar_tensor_tensor(
                        t2f[:], mx[:], -1.0, ident2[:], op0=OP.mult, op1=OP.add
                    )
                    ttp = work_ps.tile([128, 128], F32, tag="ps")
                    nc.tensor.matmul(ttp[:], t2f[:], xt32[:], start=True, stop=True)
                    nc.vector.tensor_copy(tt[:], ttp[:])
                else:
                    nc.vector.tensor_copy(tt[:], xtb[:])
                TTm[b][c] = tt

                # H' = KN Q^T (wide);  G^T = triu_incl(H')
                hp = work_ps.tile([128, 2 * C], F32, tag="ps")
                for nt in range(NT):
                    nc.tensor.matmul(
                        hp[:],
                        KT[nt][:, cs],
                        QT[nt][:, pws],
                        start=(nt == 0),
                        stop=(nt == NT - 1),
                    )
                gt = pre.tile([128, 128], F32R, tag="gt")
                nc.vector.tensor_mul(gt[:], hp[:, off : off + 128], mask_U_incl[:])
                GTm[b][c] = gt

        # =========== phase B: sequential scan, batches interleaved =========
        for c in range(NCH):
            cs = slice(c * C, (c + 1) * C)
            for b in range(BL):
                ST, KT, QT = STm[b], KTm[b], QTm[b]
                # W1 = V - KN S0^T
                kns = work_ps.tile([128, N], F32, tag="ps")
                for jt in range(NT):
                    nc.tensor.matmul(
                        kns[:],
                        KT[jt][:, cs],
                        ST[jt][:],
                        start=(jt == 0),
                        stop=(jt == NT - 1),
                    )
                w1 = seq.tile([128, N], F32R, tag="w1")
                nc.vector.scalar_tensor_tensor(
                    w1[:], kns[:], -1.0, Vnat[b][c][:], op0=OP.mult, op1=OP.add
                )
                # U = T W1
                ups = work_ps.tile([128, N], F32, tag="ps")
                nc.tensor.matmul(ups[:], TTm[b][c][:], w1[:], start=True, stop=True)
                u = seq.tile([128, N], F32R, tag="u")
                nc.vector.tensor_copy(u[:], ups[:])
                # O = Q S0^T + G U
                ops = work_ps.tile([128, N], F32, tag="ps")
                for jt in range(NT):
                    nc.tensor.matmul(
                        ops[:], QT[jt][:, cs], ST[jt][:], start=(jt == 0), stop=False
                    )
                nc.tensor.matmul(ops[:], GTm[b][c][:], u[:], start=False, stop=True)
                # out = O * silu(O)
                sl = seq.tile([128, N], F32, tag="sl")
                nc.scalar.activation(sl[:], ops[:], AT.Silu)
                og = seq.tile([128, N], F32, tag="og")
                nc.vector.tensor_mul(og[:], sl[:], ops[:])
                if rep == REPS - 1:
                    nc.sync.dma_start(out[b, cs, :], og[:])
                # S^T += KN^T U
                for jt in range(NT):
                    sup = work_ps.tile([128, N], F32, tag="ps")
                    nc.tensor.matmul(
                        sup[:],
                        Knat[b][c][:, jt * 128 : (jt + 1) * 128],
                        u[:],
                        start=True,
                        stop=True,
                    )
                    nc.vector.tensor_add(ST[jt][:], ST[jt][:], sup[:])


_CACHE: dict = {}


def _get_compiled():
    if "nc" not in _CACHE:
        nc = bacc.Bacc(
            "TRN2", target_bir_lowering=False, debug=False, num_devices=NCORES
        )
        xT = nc.dram_tensor("xT", [DIM, BL, T], F32R, kind="ExternalInput")
        wT = nc.dram_tensor("wT", [DIM, P3N], F32R, kind="ExternalInput")
        s0T = nc.dram_tensor("s0T", [BL, N, N], F32R, kind="ExternalInput")
        out = nc.dram_tensor("out", [BL, T, N], F32, kind="ExternalOutput")
        with tile.TileContext(nc) as tc, ExitStack() as ctx:
            build_body(nc, tc, ctx, xT, wT, s0T, out)
        nc.compile()
        _CACHE["nc"] = nc
    return _CACHE["nc"]


def make_in_maps(x, S0, W_k, W_v, W_q):
    x = np.ascontiguousarray(np.asarray(x, dtype=np.float32))
    S0 = np.asarray(S0, dtype=np.float32)
    wT = np.ascontiguousarray(
        np.concatenate(
            [np.asarray(W_k), np.asarray(W_v), np.asarray(W_q)], axis=0
        ).T.astype(np.float32)
    )  # [DIM, 3N]
    xT = x.transpose(2, 1, 0)  # [DIM, B, T]
    s0T = S0.transpose(0, 2, 1)  # [B, N, N] with S^T per batch
    in_maps = []
    for core in range(NCORES):
        bs = slice(core * BL, (core + 1) * BL)
        in_maps.append(
            {
                "xT": np.ascontiguousarray(xT[:, bs, :]),
                "wT": wT,
                "s0T": np.ascontiguousarray(s0T[bs]),
            }
        )
    return in_maps


def kernel(x, S0, W_k, W_v, W_q):
    nc = _get_compiled()
    in_maps = make_in_maps(x, S0, W_k, W_v, W_q)
    res = run_bass_kernel_spmd(nc, in_maps, core_ids=list(range(NCORES)))
    outs = np.concatenate([r["out"] for r in res.results], axis=0)  # [B, T, N]
    return np.ascontiguousarray(outs.transpose(1, 0, 2))  # [T, B, N]


# revision 12
# speedup vs baseline: 1.0268x; 1.0268x over previous
"""Trainium2 Bass kernel for the E74 checkpointed delta-rule cell.

Math (per batch b):
    k,v,q = x @ W_{k,v,q}^T            # [T, N] each
    kn = k / (||k|| + 1e-6)
    scan over t: r = S kn_t ; S += outer(v_t - r, kn_t) ; z = S q_t
                 out_t = z * silu(z)

Chunked parallel form (chunk C=128, per batch, chunk index c):
    A  = stril(KN KN^T)   (strict lower, within chunk)
    T  = (I + A)^{-1}     (unit lower triangular inverse, Newton iterations)
    U  = T (V - KN S0^T)
    O  = Q S0^T + (tril_incl(Q KN^T)) U
    S1^T = S0^T + KN^T U
    out = O * silu(O)

Sharding: data parallel over batch B=32 -> 8 cores x 4 batches.

Engine routing: PE does all matmuls/transposes (fp32r for big ones, bf16
for the inverse iteration, fp32 for the final Newton refinement); DVE
drains PSUM-consuming elementwise; ACT does only Copy-drains (phase A)
and Silu (phase B) to avoid act-table reloads; GPSIMD does SBUF-only
elementwise (mask builds, norm squares, Newton rsqrt for 1/(||k||+eps)).
"""

from contextlib import ExitStack

import numpy as np

import concourse.bacc as bacc
import concourse.mybir as mybir
import concourse.tile as tile
from concourse import masks
from concourse.bass_utils import run_bass_kernel_spmd

F32 = mybir.dt.float32
F32R = mybir.dt.float32r
BF16 = mybir.dt.bfloat16
I32 = mybir.dt.int32
AT = mybir.ActivationFunctionType
OP = mybir.AluOpType

T, B, DIM, N = 512, 32, 1024, 256
NCORES = 8
BL = B // NCORES  # local batches per core = 4
C = 128  # chunk length
NCH = T // C  # chunks per batch = 4
NT = N // 128  # n tiles = 2
DT = DIM // 128  # d tiles = 8
P3N = 3 * N  # packed projection width (k|v|q)

# knobs
N_BF16_ITERS = 2  # bf16 Newton iterations (order 2^(iters+1)-1 Neumann terms)
REFINE_FP32 = True  # final Newton step in fp32
RSQRT_ITERS = 3  # Newton iterations for 1/sqrt on gpsimd
REPS = 1


def build_body(nc, tc, ctx, xT, wT, s0T, out):
    consts = ctx.enter_context(tc.tile_pool(name="consts", bufs=1))
    wpool = ctx.enter_context(tc.tile_pool(name="w", bufs=1))
    xpool = ctx.enter_context(tc.tile_pool(name="x", bufs=8))
    kvq = ctx.enter_context(tc.tile_pool(name="kvq", bufs=1))
    vtp = ctx.enter_context(tc.tile_pool(name="vt", bufs=2))
    state = ctx.enter_context(tc.tile_pool(name="state", bufs=1))
    pre = ctx.enter_context(tc.tile_pool(name="pre", bufs=16))
    dbl = ctx.enter_context(tc.tile_pool(name="dbl", bufs=3))
    seq = ctx.enter_context(tc.tile_pool(name="seq", bufs=4))
    nrm = ctx.enter_context(tc.tile_pool(name="nrm", bufs=2))
    proj_ps = ctx.enter_context(tc.tile_pool(name="proj_ps", bufs=2, space="PSUM"))
    prea_ps = ctx.enter_context(tc.tile_pool(name="prea_ps", bufs=3, space="PSUM"))
    seqb_ps = ctx.enter_context(tc.tile_pool(name="seqb_ps", bufs=3, space="PSUM"))

    # ---- constants ----
    ident = consts.tile([128, 128], F32, tag="ident")
    masks.make_identity(nc, ident[:])
    ident_bf = consts.tile([128, 128], BF16, tag="identb")
    nc.gpsimd.tensor_copy(ident_bf[:], ident[:])
    ident2 = consts.tile([128, 128], F32, tag="ident2")
    masks.make_identity(nc, ident2[:])
    nc.gpsimd.tensor_scalar_mul(ident2[:], ident2[:], 2.0)
    mask_negL = consts.tile([128, 128], F32, tag="mnl")  # -1 on strict lower
    masks.make_lower_triangular(nc, mask_negL[:], val=-1.0, diag=False)
    mask_negU = consts.tile([128, 128], F32, tag="mnu")  # -1 on strict upper
    masks.make_upper_triangular(nc, mask_negU[:], val=-1.0, diag=False)
    mask_U_excl = consts.tile([128, 128], F32, tag="mue")  # 1 on strict upper
    masks.make_upper_triangular(nc, mask_U_excl[:], val=1.0, diag=False)
    mask_U_incl = consts.tile([128, 128], F32, tag="mui")  # 1 on upper+diag
    masks.make_upper_triangular(nc, mask_U_incl[:], val=1.0, diag=True)
    ident_r = consts.tile([128, 128], F32R, tag="identr")
    nc.gpsimd.tensor_copy(ident_r[:], ident[:])
    ones_col_f = consts.tile([128, 1], F32, tag="onescf")
    nc.gpsimd.memset(ones_col_f[:], 1.0)
    ones_col = consts.tile([128, 1], F32R, tag="onesc")
    nc.gpsimd.tensor_copy(ones_col[:], ones_col_f[:])
    ones_row_f = consts.tile([1, 128], F32, tag="onesrf")
    nc.gpsimd.memset(ones_row_f[:], 1.0)
    ones_row = consts.tile([1, 128], F32R, tag="onesr")
    nc.gpsimd.tensor_copy(ones_row[:], ones_row_f[:])

    # weights resident: 8 tiles [128, 768]
    w_sb = []
    for d in range(DT):
        wt = wpool.tile([128, P3N], F32R, tag=f"w{d}")
        nc.sync.dma_start(wt[:], wT[d * 128 : (d + 1) * 128, :])
        w_sb.append(wt)

    for rep in range(REPS):
        STm = [[None] * NT for _ in range(BL)]
        KTm = [[None] * NT for _ in range(BL)]
        QTm = [[None] * NT for _ in range(BL)]
        TTm = [[None] * NCH for _ in range(BL)]
        GTm = [[None] * NCH for _ in range(BL)]
        Knat = [[None] * NCH for _ in range(BL)]
        Vnat = [[None] * NCH for _ in range(BL)]

        # =========== phase A: projections + chunk precompute, per batch ====
        for b in range(BL):
            for jt in range(NT):
                st = state.tile([128, N], F32R, tag=f"st{b}{jt}")
                nc.sync.dma_start(st[:], s0T[b, jt * 128 : (jt + 1) * 128, :])
                STm[b][jt] = st

            xt = []
            for d in range(DT):
                x_sb = xpool.tile([128, T], F32R, tag="x")
                nc.sync.dma_start(x_sb[:], xT[d * 128 : (d + 1) * 128, b, :])
                xt.append(x_sb)

            # projections: out[n, t] = sum_d wT[d, n] * xT[d, t]
            KT, VT, QT = [None] * NT, [None] * NT, [None] * NT
            for p in range(3):  # k, v, q
                for nt in range(NT):
                    ps = proj_ps.tile([128, T], F32, tag="proj")
                    col0 = p * N + nt * 128
                    for d in range(DT):
                        nc.tensor.matmul(
                            ps[:],
                            w_sb[d][:, col0 : col0 + 128],
                            xt[d][:],
                            start=(d == 0),
                            stop=(d == DT - 1),
                        )
                    if p == 1:
                        dst = vtp.tile([128, T], F32R, tag=f"vt{nt}")
                    else:
                        dst = kvq.tile([128, T], F32R, tag=f"kvq{p}{nt}{b}")
                    if p == 0:
                        nc.vector.tensor_copy(dst[:], ps[:])  # k: DVE
                    else:
                        nc.scalar.copy(dst[:], ps[:])  # v, q: ACT
                    [KT, VT, QT][p][nt] = dst
            KTm[b], QTm[b] = KT, QT

            # ---- kn = k * rsqrt(ssq) (+eps folded; see note) ----
            # ssq[1, t] = sum_n k^2 via ones-vector matmul
            ssq_ps = prea_ps.tile([1, T], F32, tag="ps")
            for nt in range(NT):
                sq = nrm.tile([128, T], F32R, tag="sq")
                nc.vector.tensor_mul(sq[:], KT[nt][:], KT[nt][:])
                nc.tensor.matmul(
                    ssq_ps[:], ones_col[:], sq[:], start=(nt == 0), stop=(nt == NT - 1)
                )
            ssq = nrm.tile([1, T], F32, tag="ssq")
            nc.vector.tensor_copy(ssq[:], ssq_ps[:])
            # y0 = bit-trick seed; Newton y <- y*(1.5 - 0.5*x*y^2)
            yi = nrm.tile([1, T], I32, tag="yi")
            nc.vector.tensor_scalar(
                yi[:], ssq[:].bitcast(I32), 1, None, op0=OP.arith_shift_right
            )
            nc.vector.tensor_scalar_mul(yi[:], yi[:], -1)
            nc.vector.tensor_scalar_add(yi[:], yi[:], 0x5F3759DF)
            y = nrm.tile([1, T], F32, tag="y")
            nc.gpsimd.tensor_copy(y[:], yi[:].bitcast(F32))
            a = nrm.tile([1, T], F32, tag="a")
            for _ in range(RSQRT_ITERS):
                nc.gpsimd.tensor_mul(a[:], y[:], y[:])
                nc.gpsimd.tensor_mul(a[:], a[:], ssq[:])
                nc.gpsimd.tensor_scalar_mul(a[:], a[:], -0.5)
                nc.gpsimd.tensor_scalar_add(a[:], a[:], 1.5)
                nc.gpsimd.tensor_mul(y[:], y[:], a[:])
            # eps correction: 1/(s+eps) ~= y*(1 - eps*y); eps=1e-6, s~16
            nc.gpsimd.tensor_scalar_mul(a[:], y[:], -1e-6)
            nc.gpsimd.tensor_scalar_add(a[:], a[:], 1.0)
            inv = nrm.tile([1, T], F32R, tag="inv")
            nc.gpsimd.tensor_mul(inv[:], y[:], a[:])
            # broadcast along partitions via rank-1 matmul, then normalize k
            bc_ps = prea_ps.tile([128, T], F32, tag="ps")
            nc.tensor.matmul(bc_ps[:], ones_row[:], inv[:], start=True, stop=True)
            bc = nrm.tile([128, T], F32, tag="bc")
            nc.scalar.copy(bc[:], bc_ps[:])
            for nt in range(NT):
                nc.vector.tensor_mul(KT[nt][:], KT[nt][:], bc[:])

            # ---- per chunk precompute ----
            for c in range(NCH):
                cs = slice(c * C, (c + 1) * C)
                pw0 = c if c < NCH - 1 else c - 1
                off = 0 if c < NCH - 1 else 128
                pws = slice(pw0 * C, (pw0 + 2) * C)

                # K, V natural via PE transpose
                kn_t = pre.tile([128, N], F32R, tag="knat")
                vn_t = pre.tile([128, N], F32R, tag="vnat")
                for nt in range(NT):
                    tp = prea_ps.tile([128, 128], F32R, tag="ps")
                    nc.tensor.transpose(tp[:], KT[nt][:, cs], ident_r[:])
                    nc.scalar.copy(kn_t[:, nt * 128 : (nt + 1) * 128], tp[:])
                    tp2 = prea_ps.tile([128, 128], F32R, tag="ps")
                    nc.tensor.transpose(tp2[:], VT[nt][:, cs], ident_r[:])
                    nc.scalar.copy(vn_t[:, nt * 128 : (nt + 1) * 128], tp2[:])
                Knat[b][c] = kn_t
                Vnat[b][c] = vn_t

                # KK chunk-diagonal block (wide free=256)
                kk_ps = prea_ps.tile([128, 2 * C], F32, tag="ps")
                for nt in range(NT):
                    nc.tensor.matmul(
                        kk_ps[:],
                        KT[nt][:, cs],
                        KT[nt][:, pws],
                        start=(nt == 0),
                        stop=(nt == NT - 1),
                    )
                kkb = kk_ps[:, off : off + 128]
                lup = dbl.tile([128, 128], F32, tag="lup")
                nc.vector.tensor_mul(lup[:], kkb, mask_U_excl[:])
                nc.gpsimd.tensor_add(lup[:], lup[:], ident[:])
                lup_bf = dbl.tile([128, 128], BF16, tag="lupb")
                nc.gpsimd.tensor_copy(lup_bf[:], lup[:])
                xb = dbl.tile([128, 128], BF16, tag="xb")
                nc.vector.tensor_mul(xb[:], kkb, mask_negL[:])
                nc.gpsimd.tensor_add(xb[:], xb[:], ident_bf[:])
                xtb = dbl.tile([128, 128], BF16, tag="xtb")
                nc.vector.tensor_mul(xtb[:], kkb, mask_negU[:])
                nc.gpsimd.tensor_add(xtb[:], xtb[:], ident_bf[:])

                # bf16 Newton: X <- X(2I - M X), tracking X and X^T
                for _ in range(N_BF16_ITERS):
                    mx = prea_ps.tile([128, 128], F32, tag="ps")
                    nc.tensor.matmul(mx[:], lup_bf[:], xb[:], start=True, stop=True)
                    t2 = dbl.tile([128, 128], BF16, tag="t2")
                    nc.vector.scalar_tensor_tensor(
                        t2[:], mx[:], -1.0, ident2[:], op0=OP.mult, op1=OP.add
                    )
                    xps = prea_ps.tile([128, 128], F32, tag="ps")
                    nc.tensor.matmul(xps[:], xtb[:], t2[:], start=True, stop=True)
                    xtps = prea_ps.tile([128, 128], F32, tag="ps")
                    nc.tensor.matmul(xtps[:], t2[:], xtb[:], start=True, stop=True)
                    xb = dbl.tile([128, 128], BF16, tag="xb")
                    nc.vector.tensor_copy(xb[:], xps[:])
                    xtb = dbl.tile([128, 128], BF16, tag="xtb")
                    nc.vector.tensor_copy(xtb[:], xtps[:])

                tt = pre.tile([128, 128], F32R, tag="tt")
                if REFINE_FP32:
                    # fp32 Newton step; only T^T = (2I - M X)^T X^T needed
                    x32 = dbl.tile([128, 128], F32, tag="x32")
                    nc.gpsimd.tensor_copy(x32[:], xb[:])
                    xt32 = dbl.tile([128, 128], F32, tag="xt32")
                    nc.gpsimd.tensor_copy(xt32[:], xtb[:])
                    mx = prea_ps.tile([128, 128], F32, tag="ps")
                    nc.tensor.matmul(mx[:], lup[:], x32[:], start=True, stop=True)
                    t2f = dbl.tile([128, 128], F32, tag="t2f")
                    nc.vector.scalar_tensor_tensor(
                        t2f[:], mx[:], -1.0, ident2[:], op0=OP.mult, op1=OP.add
                    )
                    ttp = prea_ps.tile([128, 128], F32, tag="ps")
                    nc.tensor.matmul(ttp[:], t2f[:], xt32[:], start=True, stop=True)
                    nc.vector.tensor_copy(tt[:], ttp[:])
                else:
                    nc.vector.tensor_copy(tt[:], xtb[:])
                TTm[b][c] = tt

                # H' = KN Q^T (wide);  G^T = triu_incl(H')
                hp = prea_ps.tile([128, 2 * C], F32, tag="ps")
                for nt in range(NT):
                    nc.tensor.matmul(
                        hp[:],
                        KT[nt][:, cs],
                        QT[nt][:, pws],
                        start=(nt == 0),
                        stop=(nt == NT - 1),
                    )
                gt = pre.tile([128, 128], F32R, tag="gt")
                nc.vector.tensor_mul(gt[:], hp[:, off : off + 128], mask_U_incl[:])
                GTm[b][c] = gt

        # =========== phase B: sequential scan, batches interleaved =========
        for c in range(NCH):
            cs = slice(c * C, (c + 1) * C)
            for b in range(BL):
                ST, KT, QT = STm[b], KTm[b], QTm[b]
                # W1 = V - KN S0^T
                kns = seqb_ps.tile([128, N], F32, tag="ps")
                for jt in range(NT):
                    nc.tensor.matmul(
                        kns[:],
                        KT[jt][:, cs],
                        ST[jt][:],
                        start=(jt == 0),
                        stop=(jt == NT - 1),
                    )
                w1 = seq.tile([128, N], F32R, tag="w1")
                nc.vector.scalar_tensor_tensor(
                    w1[:], kns[:], -1.0, Vnat[b][c][:], op0=OP.mult, op1=OP.add
                )
                # U = T W1
                ups = seqb_ps.tile([128, N], F32, tag="ps")
                nc.tensor.matmul(ups[:], TTm[b][c][:], w1[:], start=True, stop=True)
                u = seq.tile([128, N], F32R, tag="u")
                nc.vector.tensor_copy(u[:], ups[:])
                # O = Q S0^T + G U
                ops = seqb_ps.tile([128, N], F32, tag="ps")
                for jt in range(NT):
                    nc.tensor.matmul(
                        ops[:], QT[jt][:, cs], ST[jt][:], start=(jt == 0), stop=False
                    )
                nc.tensor.matmul(ops[:], GTm[b][c][:], u[:], start=False, stop=True)
                # out = O * silu(O)
                sl = seq.tile([128, N], F32, tag="sl")
                nc.scalar.activation(sl[:], ops[:], AT.Silu)
                og = seq.tile([128, N], F32, tag="og")
                nc.vector.tensor_mul(og[:], sl[:], ops[:])
                if rep == REPS - 1:
                    nc.sync.dma_start(out[b, cs, :], og[:])
                # S^T += KN^T U
                for jt in range(NT):
                    sup = seqb_ps.tile([128, N], F32, tag="ps")
                    nc.tensor.matmul(
                        sup[:],
                        Knat[b][c][:, jt * 128 : (jt + 1) * 128],
                        u[:],
                        start=True,
                        stop=True,
                    )
                    nc.vector.tensor_add(ST[jt][:], ST[jt][:], sup[:])


_CACHE: dict = {}


def _get_compiled():
    if "nc" not in _CACHE:
        nc = bacc.Bacc(
            "TRN2", target_bir_lowering=False, debug=False, num_devices=NCORES
        )
        xT = nc.dram_tensor("xT", [DIM, BL, T], F32R, kind="ExternalInput")
        wT = nc.dram_tensor("wT", [DIM, P3N], F32R, kind="ExternalInput")
        s0T = nc.dram_tensor("s0T", [BL, N, N], F32R, kind="ExternalInput")
        out = nc.dram_tensor("out", [BL, T, N], F32, kind="ExternalOutput")
        with tile.TileContext(nc) as tc, ExitStack() as ctx:
            build_body(nc, tc, ctx, xT, wT, s0T, out)
        nc.compile()
        _CACHE["nc"] = nc
    return _CACHE["nc"]


def make_in_maps(x, S0, W_k, W_v, W_q):
    x = np.ascontiguousarray(np.asarray(x, dtype=np.float32))
    S0 = np.asarray(S0, dtype=np.float32)
    wT = np.ascontiguousarray(
        np.concatenate(
            [np.asarray(W_k), np.asarray(W_v), np.asarray(W_q)], axis=0
        ).T.astype(np.float32)
    )  # [DIM, 3N]
    xT = x.transpose(2, 1, 0)  # [DIM, B, T]
    s0T = S0.transpose(0, 2, 1)  # [B, N, N] with S^T per batch
    in_maps = []
    for core in range(NCORES):
        bs = slice(core * BL, (core + 1) * BL)
        in_maps.append(
            {
                "xT": np.ascontiguousarray(xT[:, bs, :]),
                "wT": wT,
                "s0T": np.ascontiguousarray(s0T[bs]),
            }
        )
    return in_maps


def kernel(x, S0, W_k, W_v, W_q):
    nc = _get_compiled()
    in_maps = make_in_maps(x, S0, W_k, W_v, W_q)
    res = run_bass_kernel_spmd(nc, in_maps, core_ids=list(range(NCORES)))
    outs = np.concatenate([r["out"] for r in res.results], axis=0)  # [B, T, N]
    return np.ascontiguousarray(outs.transpose(1, 0, 2))  # [T, B, N]
